# revision 34
# baseline (speedup 1.0000x reference)
"""Trainium2 Bass kernel for nn_MixtureOfExperts (argmax-routed SwiGLU MoE).

Strategy (expert-parallel across 8 NeuronCores, bf16 matmuls):
  - Host computes router logits (fp64 matmul, tiny) and the argmax expert
    per token.  Top-2 logit gaps are >=1.7e-4 while fp32 rounding noise is
    ~1e-6, so routing is insensitive to arithmetic order.
  - Each core is assigned one expert and a fixed capacity of C=512 tokens
    (zero-padded).  Tokens beyond 512 for an overloaded expert (a few tens
    out of 4096) are computed on the host in fp32 — this keeps every core
    at exactly 512 tokens (perfect balance, and C=512 means every matmul
    streams full 512-row chunks with no partition-tile waste).
  - Each core computes the SwiGLU for its tokens only:
        h = silu(x @ gw) * (x @ uw);  y = h @ dw
    in bf16 (1 PE cycle/row, same rate as fp32r, half the HBM traffic).
  - Host scatters per-core outputs back to token positions.

Layout: x is shipped pre-transposed and k-major packed ([128, KD*C],
block k = x^T[k*128:(k+1)*128, :]) so the contraction dim D lands on SBUF
partitions; mm1 produces h^T [H, C] tiles which are exactly the stationary
operand layout needed for mm2 (contraction over H).  gate/up weights are
host-packed k-major and chunk-interleaved so each weight chunk is ONE
contiguous DMA (DMA issue costs ~0.6us of sequencer time each; the whole
kernel issues ~20 DMAs instead of ~70).
"""

import numpy as np
import ml_dtypes

import concourse.mybir as mybir
import concourse.tile as tile
from concourse import bacc
from concourse.bass_utils import run_bass_kernel_spmd

B, T, D, E, H = 4, 1024, 1024, 8, 2048
BT = B * T
NCORES = 8
P = 128
KD = D // P   # k-tiles for mm1 (contraction over D)
KH = H // P   # k-tiles for mm2 (contraction over H)
C = 512       # per-core token capacity (matches PSUM bank free size)
F32 = mybir.dt.float32
BF16 = mybir.dt.bfloat16
NPBF16 = ml_dtypes.bfloat16

# gate/up weight chunks over H.  Each chunk is one contiguous DMA; DMA
# queue dispatch is per-partition-row, so rows must be >=4KB (256 H cols
# k-major) to reach full bandwidth — smaller chunks starve the queue.
H_CHUNKS = [(0, 256), (256, 256), (512, 512), (1024, 512), (1536, 512)]
assert sum(c for _, c in H_CHUNKS) == H

_BUILD_CACHE = {}

# Optional kwargs forwarded to run_bass_kernel_spmd (test harness sets
# this to enable NTFF tracing; empty for normal use).
RUN_KWARGS = {}
LAST_RESULTS = None


def _build():
    """Build the per-core SPMD Bass kernel (capacity C tokens, bf16)."""
    nc = bacc.Bacc("TRN2", target_bir_lowering=False, debug=False)
    # k-major packed operands: block k of xt is x^T[k*128:(k+1)*128, :C]
    xt = nc.dram_tensor("xt", [P, KD * C], BF16, kind="ExternalInput")
    gu = nc.dram_tensor("gu", [P, KD * 2 * H], BF16, kind="ExternalInput")
    dw = nc.dram_tensor("dw", [P, KH * D], BF16, kind="ExternalInput")
    y = nc.dram_tensor("y", [C, D], F32, kind="ExternalOutput")

    with tile.TileContext(nc) as tc:
        with (
            tc.tile_pool(name="xp", bufs=1) as xp,
            tc.tile_pool(name="dmp", bufs=1) as dmp,
            tc.tile_pool(name="hp", bufs=KH) as hp,
            tc.tile_pool(name="w1s", bufs=3) as w1s,
            tc.tile_pool(name="w1b", bufs=3) as w1b,
            tc.tile_pool(name="w2", bufs=2) as w2,
            tc.tile_pool(name="outp", bufs=3) as outp,
            tc.tile_pool(name="ps", bufs=8, space="PSUM") as ps,
        ):
            # DMA issue costs ~0.6us of serialized sequencer time per
            # dma_start.  Only SP (sync) and Activation (scalar) have
            # hardware descriptor generation — gpsimd falls back to slow
            # SWDGE, so keep it out.  At the head both engines' queues are
            # empty; interleave the critical prefix across them.
            _eng_i = [0]

            def dma(dst, src, eng=None):
                engs = [nc.sync, nc.scalar]
                (eng or engs[_eng_i[0] % 2]).dma_start(dst, src)
                _eng_i[0] += 1

            # PE p-state warmup: the Tensor engine ramps 0.65 -> 1.2 ->
            # 2.4GHz over ~3us of continuous execution.  While the head
            # DMAs are in flight the PE is idle, so run dep-free dummy
            # matmuls on a memset tile — by the time real data lands the
            # PE is at full clock and stays there (512-row matmuls take
            # 216ns instead of 427ns for the first ~3us).
            dmy = dmp.tile([P, C], BF16, tag="dmy")
            nc.gpsimd.memset(dmy[:], 0.0)
            pdmy = ps.tile([P, C], F32, tag="ps", name="pdmy")
            for _ in range(42):
                nc.tensor.matmul(pdmy[:, :], lhsT=dmy[:, :P], rhs=dmy[:],
                                 start=True, stop=True)

            # Critical prefix: the early window is fabric-bandwidth-bound
            # (all 8 cores pull at once, ~150-250GB/s effective per core),
            # so only x + the first two gate/up chunks are issued up front;
            # everything else is paced by mm1 progress.
            xa = xp.tile([P, KD * C], BF16, tag="x")
            nc.sync.dma_start(xa[:], xt[:])
            x_t = [xa[:, k * C:(k + 1) * C] for k in range(KD)]

            # gate/up chunks: ONE contiguous DMA per H-chunk (k-major
            # packed on host).  Within chunk ci at column base,
            # slice (k, hs, gate) = [base + k*2hcn + hs*P : +P]
            #       (k, hs, up)   = [base + k*2hcn + hcn + hs*P : +P]
            w_t = []
            col = 0
            for ci, (hc0, hcn) in enumerate(H_CHUNKS):
                ncols = KD * 2 * hcn
                pool = w1s if hcn <= 256 else w1b
                t = pool.tile([P, KD * 2 * hcn], BF16, tag=pool.name)
                if ci == 0:
                    # partition-split (32-aligned slices only — misaligned
                    # splits poison DMA efficiency): scalar takes the bulk;
                    # gpsimd's slower SWDGE queue handles a 32-partition
                    # slice concurrently so sync stays dedicated to x.
                    nc.scalar.dma_start(t[:96, :], gu[:96, col:col + ncols])
                    nc.gpsimd.dma_start(t[96:, :], gu[96:, col:col + ncols])
                elif ci == 1:
                    nc.scalar.dma_start(t[:96, :], gu[:96, col:col + ncols])
                    nc.gpsimd.dma_start(t[96:, :], gu[96:, col:col + ncols])
                # ci >= 2: issued later, paced by mm1 progress (below)
                w_t.append(t)
                col += KD * 2 * hcn

            w_cols = []
            col = 0
            for hc0, hcn in H_CHUNKS:
                w_cols.append(col)
                col += KD * 2 * hcn

            # down-proj weight tiles; DMAs paced by mm1 progress (below).
            dwt = [w2.tile([P, (KH // 2) * D], BF16, tag="w2",
                           name=f"dw{i}") for i in range(2)]

            # Just-in-time weight loads: with all 8 cores bursting their
            # full 14.4MB at launch, aggregate demand (~3.3TB/s) exceeds
            # the chip fabric and the slowest core's head stretches.
            # Issuing the later chunks from scalar's instruction stream
            # between mm1 activations spreads the traffic over the mm1
            # window; each chunk still lands ~10us before it is consumed.
            def paced_loads(cn):
                if cn == 1:
                    nc.sync.dma_start(w_t[2][:],
                                      gu[:, w_cols[2]:w_cols[2] + KD * 2 * 512])
                elif cn == 3:
                    nc.scalar.dma_start(w_t[3][:],
                                        gu[:, w_cols[3]:w_cols[3] + KD * 2 * 512])
                elif cn == 6:
                    nc.scalar.dma_start(w_t[4][:],
                                        gu[:, w_cols[4]:w_cols[4] + KD * 2 * 512])
                elif cn == 9:
                    nc.scalar.dma_start(dwt[0][:], dw[:, :(KH // 2) * D])
                elif cn == 11:
                    nc.scalar.dma_start(dwt[1][:], dw[:, (KH // 2) * D:])

            def dw_slice(k, nd0, ndn):
                t = dwt[k // (KH // 2)]
                base = (k % (KH // 2)) * D
                return t[:, base + nd0:base + nd0 + ndn]

            # ---- mm1: hT[j] = silu(gw.T x) * (uw.T x), tiled over H ----
            h_t = []
            for ci, (hc0, hcn) in enumerate(H_CHUNKS):
                wt = w_t[ci][:]
                for hs in range(hcn // P):
                    ht = hp.tile([P, C], BF16, tag="h")
                    pa = ps.tile([P, C], F32, tag="ps", name="pa")
                    pu = ps.tile([P, C], F32, tag="ps", name="pu")
                    for k in range(KD):
                        nc.tensor.matmul(
                            pa[:, :],
                            lhsT=wt[:, k * 2 * hcn + hs * P:
                                    k * 2 * hcn + hs * P + P],
                            rhs=x_t[k],
                            start=(k == 0), stop=(k == KD - 1),
                        )
                        nc.tensor.matmul(
                            pu[:, :],
                            lhsT=wt[:, k * 2 * hcn + hcn + hs * P:
                                    k * 2 * hcn + hcn + hs * P + P],
                            rhs=x_t[k],
                            start=(k == 0), stop=(k == KD - 1),
                        )
                    nc.scalar.activation(
                        ht[:, :], pa[:, :],
                        mybir.ActivationFunctionType.Silu,
                    )
                    nc.vector.tensor_mul(ht[:, :], ht[:, :], pu[:, :])
                    h_t.append(ht)
                    paced_loads(len(h_t))

            # ---- mm2: y = h @ dw, contraction over H ----
            for nd0 in range(0, D, C):
                for m in range(C // P):
                    py = ps.tile([P, C], F32, tag="ps", name="py")
                    for k in range(KH):
                        nc.tensor.matmul(
                            py[:, :],
                            lhsT=h_t[k][:, m * P:(m + 1) * P],
                            rhs=dw_slice(k, nd0, C),
                            start=(k == 0),
                            stop=(k == KH - 1),
                        )
                    # Drain PSUM in two half-tiles so the DMA of the first
                    # half overlaps the copy of the second (shortens the
                    # critical tail after the last matmul).  The final
                    # half rides both queues (partition-split) to halve
                    # its packet-bound drain time.
                    last = (nd0 == D - C) and (m == C // P - 1)
                    ot = outp.tile([P, C], F32, tag="out")
                    for h0 in range(0, C, C // 2):
                        nc.vector.tensor_copy(ot[:, h0:h0 + C // 2],
                                              py[:, h0:h0 + C // 2])
                        cols = slice(nd0 + h0, nd0 + h0 + C // 2)
                        if last and h0:
                            nc.sync.dma_start(
                                y[m * P:m * P + 64, cols], ot[:64, h0:h0 + C // 2])
                            nc.scalar.dma_start(
                                y[m * P + 64:(m + 1) * P, cols],
                                ot[64:, h0:h0 + C // 2])
                        else:
                            dma(y[m * P:(m + 1) * P, cols],
                                ot[:, h0:h0 + C // 2])

    nc.compile()
    return nc


def _get_kernel():
    if "k" not in _BUILD_CACHE:
        _BUILD_CACHE["k"] = _build()
    return _BUILD_CACHE["k"]


def _route(xf, gate_w):
    """argmax expert per token, computed in fp64 on host (negligible work)."""
    logits = xf.astype(np.float64) @ np.asarray(gate_w, np.float64).T
    return logits.argmax(axis=1)


def _bf16(a):
    return np.ascontiguousarray(np.asarray(a, np.float32)).astype(NPBF16)


def _pack_gu(gw_e, uw_e):
    """k-major chunk-interleaved [P, KD*2H]: chunk ci holds KD blocks of
    [gate[kP:(k+1)P, hc0:hc0+hcn] | up[...]]."""
    parts = []
    for hc0, hcn in H_CHUNKS:
        for k in range(KD):
            parts.append(gw_e[k * P:(k + 1) * P, hc0:hc0 + hcn])
            parts.append(uw_e[k * P:(k + 1) * P, hc0:hc0 + hcn])
    return np.ascontiguousarray(np.concatenate(parts, axis=1))


def _pack_k_major(a):
    """[R*P, N] -> [P, R*N] with block r = a[r*P:(r+1)*P, :]."""
    r = a.shape[0] // P
    return np.ascontiguousarray(
        a.reshape(r, P, a.shape[1]).transpose(1, 0, 2).reshape(P, -1))


def _silu_swiglu_host(xo, gw, uw, dwn):
    """fp32 reference path for host-computed overflow tokens."""
    a = xo @ gw
    u = xo @ uw
    h = u * (a / (1.0 + np.exp(-a)))
    return h @ dwn


def kernel(x, gate_w, gate_bank, up_bank, down_bank):
    global LAST_RESULTS
    x = np.asarray(x, np.float32)
    assert x.shape == (B, T, D)

    xf = np.ascontiguousarray(x.reshape(BT, D))
    sel = _route(xf, gate_w)
    idx = [np.nonzero(sel == e)[0] for e in range(E)]
    keep = [i[:C] for i in idx]
    over = [i[C:] for i in idx]

    nc = _get_kernel()

    gate_bank = np.asarray(gate_bank, np.float32)
    up_bank = np.asarray(up_bank, np.float32)
    down_bank = np.asarray(down_bank, np.float32)
    gb16 = _bf16(gate_bank)
    ub16 = _bf16(up_bank)
    db16 = _bf16(down_bank)
    x16 = _bf16(xf)

    in_maps = []
    for e in range(E):
        xe = np.zeros((D, C), NPBF16)
        n = len(keep[e])
        if n:
            xe[:, :n] = x16[keep[e]].T
        in_maps.append({
            "xt": _pack_k_major(xe),
            "gu": _pack_gu(gb16[e], ub16[e]),
            "dw": _pack_k_major(db16[e]),
        })

    res = run_bass_kernel_spmd(nc, in_maps, core_ids=list(range(NCORES)),
                               **RUN_KWARGS)
    LAST_RESULTS = res

    out = np.empty((BT, D), np.float32)
    for e in range(E):
        n = len(keep[e])
        if n:
            out[keep[e]] = res.results[e]["y"][:n]
        if len(over[e]):
            out[over[e]] = _silu_swiglu_host(
                xf[over[e]], gate_bank[e], up_bank[e], down_bank[e])
    return out.reshape(B, T, D)


# revision 35
# speedup vs baseline: 1.0499x; 1.0499x over previous
"""Trainium2 Bass kernel for nn_MixtureOfExperts (argmax-routed SwiGLU MoE).

Strategy (expert-parallel across 8 NeuronCores, bf16 matmuls):
  - Host computes router logits (fp64 matmul, tiny) and the argmax expert
    per token.  Top-2 logit gaps are >=1.7e-4 while fp32 rounding noise is
    ~1e-6, so routing is insensitive to arithmetic order.
  - Each core is assigned one expert and a fixed capacity of C=512 tokens
    (zero-padded).  Tokens beyond 512 for an overloaded expert (a few tens
    out of 4096) are computed on the host in fp32 — this keeps every core
    at exactly 512 tokens (perfect balance, and C=512 means every matmul
    streams full 512-row chunks with no partition-tile waste).
  - Each core computes the SwiGLU for its tokens only:
        h = silu(x @ gw) * (x @ uw);  y = h @ dw
    in bf16 (1 PE cycle/row, same rate as fp32r, half the HBM traffic).
  - Host scatters per-core outputs back to token positions.

Layout: x is shipped pre-transposed and k-major packed ([128, KD*C],
block k = x^T[k*128:(k+1)*128, :]) so the contraction dim D lands on SBUF
partitions; mm1 produces h^T [H, C] tiles which are exactly the stationary
operand layout needed for mm2 (contraction over H).  gate/up weights are
host-packed k-major and chunk-interleaved so each weight chunk is ONE
contiguous DMA (DMA issue costs ~0.6us of sequencer time each; the whole
kernel issues ~20 DMAs instead of ~70).
"""

import numpy as np
import ml_dtypes

import concourse.mybir as mybir
import concourse.tile as tile
from concourse import bacc
from concourse.bass_utils import run_bass_kernel_spmd

B, T, D, E, H = 4, 1024, 1024, 8, 2048
BT = B * T
NCORES = 8
P = 128
KD = D // P   # k-tiles for mm1 (contraction over D)
KH = H // P   # k-tiles for mm2 (contraction over H)
C = 512       # per-core token capacity (matches PSUM bank free size)
F32 = mybir.dt.float32
BF16 = mybir.dt.bfloat16
NPBF16 = ml_dtypes.bfloat16

# gate/up weight chunks over H.  Each chunk is one contiguous DMA; DMA
# queue dispatch is per-partition-row, so rows must be >=4KB (256 H cols
# k-major) to reach full bandwidth — smaller chunks starve the queue.
H_CHUNKS = [(0, 256), (256, 256), (512, 512), (1024, 512), (1536, 512)]
assert sum(c for _, c in H_CHUNKS) == H

_BUILD_CACHE = {}

# Optional kwargs forwarded to run_bass_kernel_spmd (test harness sets
# this to enable NTFF tracing; empty for normal use).
RUN_KWARGS = {}
LAST_RESULTS = None


def _build():
    """Build the per-core SPMD Bass kernel (capacity C tokens, bf16)."""
    nc = bacc.Bacc("TRN2", target_bir_lowering=False, debug=False)
    # k-major packed operands: block k of xt is x^T[k*128:(k+1)*128, :C]
    xt = nc.dram_tensor("xt", [P, KD * C], BF16, kind="ExternalInput")
    gu = nc.dram_tensor("gu", [P, KD * 2 * H], BF16, kind="ExternalInput")
    dw = nc.dram_tensor("dw", [P, KH * D], BF16, kind="ExternalInput")
    y = nc.dram_tensor("y", [C, D], F32, kind="ExternalOutput")

    with tile.TileContext(nc) as tc:
        with (
            tc.tile_pool(name="xp", bufs=1) as xp,
            tc.tile_pool(name="dmp", bufs=1) as dmp,
            tc.tile_pool(name="hp", bufs=KH) as hp,
            tc.tile_pool(name="w1s", bufs=3) as w1s,
            tc.tile_pool(name="w1b", bufs=3) as w1b,
            tc.tile_pool(name="w2", bufs=2) as w2,
            tc.tile_pool(name="outp", bufs=3) as outp,
            tc.tile_pool(name="ps", bufs=8, space="PSUM") as ps,
        ):
            # DMA issue costs ~0.6us of serialized sequencer time per
            # dma_start.  Only SP (sync) and Activation (scalar) have
            # hardware descriptor generation — gpsimd falls back to slow
            # SWDGE, so keep it out.  At the head both engines' queues are
            # empty; interleave the critical prefix across them.
            _eng_i = [0]

            def dma(dst, src, eng=None):
                engs = [nc.sync, nc.scalar]
                (eng or engs[_eng_i[0] % 2]).dma_start(dst, src)
                _eng_i[0] += 1

            # PE p-state warmup: the Tensor engine ramps 0.65 -> 1.2 ->
            # 2.4GHz over ~3us of continuous execution.  While the head
            # DMAs are in flight the PE is idle, so run dep-free dummy
            # matmuls on a memset tile — by the time real data lands the
            # PE is at full clock and stays there (512-row matmuls take
            # 216ns instead of 427ns for the first ~3us).
            dmy = dmp.tile([P, C], BF16, tag="dmy")
            nc.gpsimd.memset(dmy[:], 0.0)
            pdmy = ps.tile([P, C], F32, tag="ps", name="pdmy")
            for _ in range(42):
                nc.tensor.matmul(pdmy[:, :], lhsT=dmy[:, :P], rhs=dmy[:],
                                 start=True, stop=True)

            # Critical prefix: the early window is fabric-bandwidth-bound
            # (all 8 cores pull at once, ~150-250GB/s effective per core),
            # so only x + the first two gate/up chunks are issued up front;
            # everything else is paced by mm1 progress.
            xa = xp.tile([P, KD * C], BF16, tag="x")
            nc.sync.dma_start(xa[:], xt[:])
            x_t = [xa[:, k * C:(k + 1) * C] for k in range(KD)]

            # gate/up chunks: ONE contiguous DMA per H-chunk (k-major
            # packed on host).  Within chunk ci at column base,
            # slice (k, hs, gate) = [base + k*2hcn + hs*P : +P]
            #       (k, hs, up)   = [base + k*2hcn + hcn + hs*P : +P]
            w_t = []
            col = 0
            for ci, (hc0, hcn) in enumerate(H_CHUNKS):
                ncols = KD * 2 * hcn
                pool = w1s if hcn <= 256 else w1b
                t = pool.tile([P, KD * 2 * hcn], BF16, tag=pool.name)
                if ci == 0:
                    # partition-split (32-aligned slices only — misaligned
                    # splits poison DMA efficiency): scalar takes the bulk;
                    # gpsimd's slower SWDGE queue handles a 32-partition
                    # slice concurrently so sync stays dedicated to x.
                    nc.scalar.dma_start(t[:96, :], gu[:96, col:col + ncols])
                    nc.gpsimd.dma_start(t[96:, :], gu[96:, col:col + ncols])
                elif ci == 1:
                    nc.scalar.dma_start(t[:96, :], gu[:96, col:col + ncols])
                    nc.sync.dma_start(t[96:, :], gu[96:, col:col + ncols])
                elif ci == 2:
                    nc.sync.dma_start(t[:], gu[:, col:col + ncols])
                # ci >= 3: issued later, paced by mm1 progress (below)
                w_t.append(t)
                col += KD * 2 * hcn

            w_cols = []
            col = 0
            for hc0, hcn in H_CHUNKS:
                w_cols.append(col)
                col += KD * 2 * hcn

            # down-proj weight tiles; DMAs paced by mm1 progress (below).
            dwt = [w2.tile([P, (KH // 2) * D], BF16, tag="w2",
                           name=f"dw{i}") for i in range(2)]

            # Just-in-time weight loads: with all 8 cores bursting their
            # full 14.4MB at launch, aggregate demand (~3.3TB/s) exceeds
            # the chip fabric and the slowest core's head stretches.
            # Issuing the later chunks from scalar's instruction stream
            # between mm1 activations spreads the traffic over the mm1
            # window; each chunk still lands ~10us before it is consumed.
            def paced_loads(cn):
                if cn == 2:
                    nc.scalar.dma_start(w_t[3][:],
                                        gu[:, w_cols[3]:w_cols[3] + KD * 2 * 512])
                elif cn == 6:
                    nc.scalar.dma_start(w_t[4][:],
                                        gu[:, w_cols[4]:w_cols[4] + KD * 2 * 512])
                elif cn == 8:
                    nc.scalar.dma_start(dwt[0][:], dw[:, :(KH // 2) * D])
                elif cn == 10:
                    nc.scalar.dma_start(dwt[1][:], dw[:, (KH // 2) * D:])

            def dw_slice(k, nd0, ndn):
                t = dwt[k // (KH // 2)]
                base = (k % (KH // 2)) * D
                return t[:, base + nd0:base + nd0 + ndn]

            # ---- mm1: hT[j] = silu(gw.T x) * (uw.T x), tiled over H ----
            h_t = []
            for ci, (hc0, hcn) in enumerate(H_CHUNKS):
                wt = w_t[ci][:]
                for hs in range(hcn // P):
                    ht = hp.tile([P, C], BF16, tag="h")
                    pa = ps.tile([P, C], F32, tag="ps", name="pa")
                    pu = ps.tile([P, C], F32, tag="ps", name="pu")
                    for k in range(KD):
                        nc.tensor.matmul(
                            pa[:, :],
                            lhsT=wt[:, k * 2 * hcn + hs * P:
                                    k * 2 * hcn + hs * P + P],
                            rhs=x_t[k],
                            start=(k == 0), stop=(k == KD - 1),
                        )
                        nc.tensor.matmul(
                            pu[:, :],
                            lhsT=wt[:, k * 2 * hcn + hcn + hs * P:
                                    k * 2 * hcn + hcn + hs * P + P],
                            rhs=x_t[k],
                            start=(k == 0), stop=(k == KD - 1),
                        )
                    nc.scalar.activation(
                        ht[:, :], pa[:, :],
                        mybir.ActivationFunctionType.Silu,
                    )
                    nc.vector.tensor_mul(ht[:, :], ht[:, :], pu[:, :])
                    h_t.append(ht)
                    paced_loads(len(h_t))

            # ---- mm2: y = h @ dw, contraction over H ----
            for nd0 in range(0, D, C):
                for m in range(C // P):
                    py = ps.tile([P, C], F32, tag="ps", name="py")
                    for k in range(KH):
                        nc.tensor.matmul(
                            py[:, :],
                            lhsT=h_t[k][:, m * P:(m + 1) * P],
                            rhs=dw_slice(k, nd0, C),
                            start=(k == 0),
                            stop=(k == KH - 1),
                        )
                    # Drain PSUM -> SBUF -> DRAM.  Ordinary tiles drain
                    # whole (fewer packets + semaphores); the final tile
                    # drains in two halves with the last half partition-
                    # split across both queues, shortening the critical
                    # tail after the last matmul.
                    last = (nd0 == D - C) and (m == C // P - 1)
                    ot = outp.tile([P, C], F32, tag="out")
                    if not last:
                        nc.vector.tensor_copy(ot[:, :], py[:, :])
                        dma(y[m * P:(m + 1) * P, nd0:nd0 + C], ot[:, :])
                    else:
                        for h0 in range(0, C, C // 2):
                            nc.vector.tensor_copy(ot[:, h0:h0 + C // 2],
                                                  py[:, h0:h0 + C // 2])
                            cols = slice(nd0 + h0, nd0 + h0 + C // 2)
                            if h0:
                                nc.sync.dma_start(y[m * P:m * P + 64, cols],
                                                  ot[:64, h0:h0 + C // 2])
                                nc.scalar.dma_start(
                                    y[m * P + 64:(m + 1) * P, cols],
                                    ot[64:, h0:h0 + C // 2])
                            else:
                                dma(y[m * P:(m + 1) * P, cols],
                                    ot[:, h0:h0 + C // 2])

    nc.compile()
    return nc


def _get_kernel():
    if "k" not in _BUILD_CACHE:
        _BUILD_CACHE["k"] = _build()
    return _BUILD_CACHE["k"]


def _route(xf, gate_w):
    """argmax expert per token, computed in fp64 on host (negligible work)."""
    logits = xf.astype(np.float64) @ np.asarray(gate_w, np.float64).T
    return logits.argmax(axis=1)


def _bf16(a):
    return np.ascontiguousarray(np.asarray(a, np.float32)).astype(NPBF16)


def _pack_gu(gw_e, uw_e):
    """k-major chunk-interleaved [P, KD*2H]: chunk ci holds KD blocks of
    [gate[kP:(k+1)P, hc0:hc0+hcn] | up[...]]."""
    parts = []
    for hc0, hcn in H_CHUNKS:
        for k in range(KD):
            parts.append(gw_e[k * P:(k + 1) * P, hc0:hc0 + hcn])
            parts.append(uw_e[k * P:(k + 1) * P, hc0:hc0 + hcn])
    return np.ascontiguousarray(np.concatenate(parts, axis=1))


def _pack_k_major(a):
    """[R*P, N] -> [P, R*N] with block r = a[r*P:(r+1)*P, :]."""
    r = a.shape[0] // P
    return np.ascontiguousarray(
        a.reshape(r, P, a.shape[1]).transpose(1, 0, 2).reshape(P, -1))


def _silu_swiglu_host(xo, gw, uw, dwn):
    """fp32 reference path for host-computed overflow tokens."""
    a = xo @ gw
    u = xo @ uw
    h = u * (a / (1.0 + np.exp(-a)))
    return h @ dwn


def kernel(x, gate_w, gate_bank, up_bank, down_bank):
    global LAST_RESULTS
    x = np.asarray(x, np.float32)
    assert x.shape == (B, T, D)

    xf = np.ascontiguousarray(x.reshape(BT, D))
    sel = _route(xf, gate_w)
    idx = [np.nonzero(sel == e)[0] for e in range(E)]
    keep = [i[:C] for i in idx]
    over = [i[C:] for i in idx]

    nc = _get_kernel()

    gate_bank = np.asarray(gate_bank, np.float32)
    up_bank = np.asarray(up_bank, np.float32)
    down_bank = np.asarray(down_bank, np.float32)
    gb16 = _bf16(gate_bank)
    ub16 = _bf16(up_bank)
    db16 = _bf16(down_bank)
    x16 = _bf16(xf)

    in_maps = []
    for e in range(E):
        xe = np.zeros((D, C), NPBF16)
        n = len(keep[e])
        if n:
            xe[:, :n] = x16[keep[e]].T
        in_maps.append({
            "xt": _pack_k_major(xe),
            "gu": _pack_gu(gb16[e], ub16[e]),
            "dw": _pack_k_major(db16[e]),
        })

    res = run_bass_kernel_spmd(nc, in_maps, core_ids=list(range(NCORES)),
                               **RUN_KWARGS)
    LAST_RESULTS = res

    out = np.empty((BT, D), np.float32)
    for e in range(E):
        n = len(keep[e])
        if n:
            out[keep[e]] = res.results[e]["y"][:n]
        if len(over[e]):
            out[over[e]] = _silu_swiglu_host(
                xf[over[e]], gate_bank[e], up_bank[e], down_bank[e])
    return out.reshape(B, T, D)
